# revision 21
# baseline (speedup 1.0000x reference)
"""AttractorGenerator kernel for 8 TRN2 NeuronCores.

Strategy:
  - Algebraic fusion (host): ctx = X@W_ctx + b_ctx is folded into the K/V
    projections:  K = X @ (W_ctx@Wk) + (b_ctx@Wk + bk),  same for V.
    h0 = mean(X,0) @ W_ctx + b_ctx.
  - Device (SPMD over 8 cores): each core computes its frame-shard of
    [K^T | V^T] = [Wk' | Wv']^T-stationary matmuls streaming X^T.  This is
    the dominant compute/memory phase (103 GFLOP, 134 MB of activations).
    I/O is bf16 (fp32 PSUM accumulate); weight blocks are one contiguous
    [128,128] tile each so the bf16 fast-weight-load path stays legal.
  - The 16-step sequential GRU decode (tiny per-step math over the
    precomputed K/V) runs on host in float32.
"""

import numpy as np

N, IN_DIM, H, ATTR, HEADS = 65536, 512, 512, 256, 8
HD = H // HEADS
NCORES = 8
SHARD = N // NCORES  # 8192

_NC = None  # compiled bass program cache


def _ensure_path():
    import sys

    for p in ("/opt/trn_rl_repo", "/root/.axon_site/_ro/trn_rl_repo"):
        try:
            import concourse  # noqa: F401

            return
        except Exception:
            if p not in sys.path:
                sys.path.insert(0, p)


def _build_bass():
    _ensure_path()
    import concourse.bass as bass
    import concourse.tile as tile
    from concourse import bacc, mybir

    f32 = mybir.dt.float32
    bf16 = mybir.dt.bfloat16
    nc = bacc.Bacc(
        "TRN2",
        target_bir_lowering=False,
        debug=False,
        num_devices=NCORES,
    )
    xT = nc.dram_tensor("xT", [IN_DIM, SHARD], bf16, kind="ExternalInput").ap()
    wkv = nc.dram_tensor("wkv", [IN_DIM, 2 * H], bf16, kind="ExternalInput").ap()
    kvT = nc.dram_tensor("kvT", [2 * H, SHARD], bf16, kind="ExternalOutput").ap()

    KT = IN_DIM // 128  # 4 contraction tiles
    MT = (2 * H) // 128  # 8 output-feature tiles
    NCH = SHARD // 512  # 16 frame chunks per shard

    with tile.TileContext(nc) as tc:
        with (
            tc.tile_pool(name="x", bufs=1) as xpool,
            tc.tile_pool(name="w", bufs=1) as wpool,
            tc.tile_pool(name="st", bufs=6) as stpool,
            tc.tile_pool(name="ps", bufs=6, space=bass.MemorySpace.PSUM) as pspool,
        ):
            # one contiguous [128,128] tile per (k, m) weight block — a strided
            # slice of a wider tile breaks the bf16 fast-weight-load path
            wtiles = {}
            for k in range(KT):
                for m in range(MT):
                    t = wpool.tile([128, 128], bf16, tag=f"w{k}_{m}")
                    nc.sync.dma_start(
                        t[:], wkv[k * 128 : (k + 1) * 128, m * 128 : (m + 1) * 128]
                    )
                    wtiles[(k, m)] = t
            xtiles = []
            for k in range(KT):
                t = xpool.tile([128, SHARD], bf16, tag=f"x{k}")
                nc.sync.dma_start(t[:], xT[k * 128 : (k + 1) * 128, :])
                xtiles.append(t)
            for m in range(MT):
                for c in range(NCH):
                    ps = pspool.tile([128, 512], f32)
                    for k in range(KT):
                        nc.tensor.matmul(
                            ps[:],
                            wtiles[(k, m)][:],
                            xtiles[k][:, c * 512 : (c + 1) * 512],
                            start=(k == 0),
                            stop=(k == KT - 1),
                        )
                    st = stpool.tile([128, 512], bf16)
                    nc.vector.tensor_copy(st[:], ps[:])
                    nc.sync.dma_start(
                        kvT[m * 128 : (m + 1) * 128, c * 512 : (c + 1) * 512], st[:]
                    )
    nc.compile()
    return nc


def _run_device(X, wkv_np, trace=False):
    """Returns (list of per-core kvT [2H, SHARD] arrays, exec_time_ns or None)."""
    global _NC
    _ensure_path()
    from concourse.bass_utils import run_bass_kernel_spmd

    if _NC is None:
        _NC = _build_bass()
    import ml_dtypes

    bf = ml_dtypes.bfloat16
    wkv_bf = np.ascontiguousarray(wkv_np.astype(bf))
    in_maps = []
    for i in range(NCORES):
        xt = np.ascontiguousarray(X[i * SHARD : (i + 1) * SHARD].T.astype(bf))
        in_maps.append({"xT": xt, "wkv": wkv_bf})
    global _LAST_DEV_WALL_NS
    import time as _time

    t0 = _time.time()
    try:
        br = run_bass_kernel_spmd(_NC, in_maps, list(range(NCORES)), trace=trace)
    except Exception:
        if not trace:
            raise
        br = run_bass_kernel_spmd(_NC, in_maps, list(range(NCORES)), trace=False)
    _LAST_DEV_WALL_NS = int((_time.time() - t0) * 1e9)
    outs = [r["kvT"] for r in br.results]
    return outs, getattr(br, "exec_time_ns", None)


_LAST_EXEC_NS = None
_LAST_DEV_WALL_NS = None


def kernel(
    frame_embeddings,
    W_ctx,
    b_ctx,
    Wq,
    bq,
    Wk,
    bk,
    Wv,
    bv,
    Wo,
    bo,
    W_ih,
    W_hh,
    b_ih,
    b_hn,
    start_token,
    W_attr,
    b_attr,
    W_conf,
    b_conf,
    num_attractors,
    **_unused,
):
    global _LAST_EXEC_NS
    f = np.float32
    X = np.asarray(frame_embeddings, f)
    W_ctx = np.asarray(W_ctx, f)
    b_ctx = np.asarray(b_ctx, f)
    Wk_f = W_ctx @ np.asarray(Wk, f)
    Wv_f = W_ctx @ np.asarray(Wv, f)
    bk_f = b_ctx @ np.asarray(Wk, f) + np.asarray(bk, f)
    bv_f = b_ctx @ np.asarray(Wv, f) + np.asarray(bv, f)
    wkv_np = np.ascontiguousarray(np.concatenate([Wk_f, Wv_f], axis=1))  # [512,1024]

    trace = bool(int(__import__("os").environ.get("KERNEL_TRACE", "0")))
    outs, _LAST_EXEC_NS = _run_device(X, wkv_np, trace=trace)

    # assemble K^T, V^T over the full frame axis: [H, N]
    kT = np.concatenate([np.asarray(o[:H, :], f) for o in outs], axis=1) + bk_f[:, None]
    vT = np.concatenate([np.asarray(o[H:, :], f) for o in outs], axis=1) + bv_f[:, None]
    kT = np.ascontiguousarray(kT, f)
    vT = np.ascontiguousarray(vT, f)

    h0 = (X.mean(axis=0) @ W_ctx + b_ctx).astype(f)

    Wq = np.asarray(Wq, f)
    bq = np.asarray(bq, f)
    Wo = np.asarray(Wo, f)
    bo = np.asarray(bo, f)
    W_ih = np.asarray(W_ih, f)
    W_hh = np.asarray(W_hh, f)
    b_ih = np.asarray(b_ih, f)
    b_hn = np.asarray(b_hn, f)
    start_tok = np.asarray(start_token, f)
    W_attr = np.asarray(W_attr, f)
    b_attr = np.asarray(b_attr, f)
    W_conf = np.asarray(W_conf, f)
    b_conf = np.asarray(b_conf, f)

    K = int(np.asarray(num_attractors))
    scale = f(1.0) / np.sqrt(f(HD))

    def sigmoid(x):
        return 1.0 / (1.0 + np.exp(-x))

    kT_h = kT.reshape(HEADS, HD, N)
    vT_h = vT.reshape(HEADS, HD, N)

    h = h0
    prev_a = start_tok
    attractors = np.zeros((K, ATTR), f)
    confidences = np.zeros((K,), f)
    for t in range(K):
        q = (h @ Wq + bq).reshape(HEADS, HD)
        context = np.empty((HEADS, HD), f)
        for hh_ in range(HEADS):
            logits = (q[hh_] @ kT_h[hh_]) * scale  # [N]
            m = logits.max()
            p = np.exp(logits - m)
            p /= p.sum()
            context[hh_] = vT_h[hh_] @ p
        context = context.reshape(H) @ Wo + bo
        x = np.concatenate([prev_a, context]).astype(f)
        ih = W_ih @ x + b_ih
        hhv = W_hh @ h
        r = sigmoid(ih[:H] + hhv[:H])
        z = sigmoid(ih[H : 2 * H] + hhv[H : 2 * H])
        n_ = np.tanh(ih[2 * H :] + r * (hhv[2 * H :] + b_hn))
        h = ((1.0 - z) * n_ + z * h).astype(f)
        a = W_attr @ h + b_attr
        c = sigmoid(W_conf @ h + b_conf)[0]
        attractors[t] = a
        confidences[t] = c
        prev_a = a

    return attractors, confidences
